# revision 9
# baseline (speedup 1.0000x reference)
"""GAT 2-layer GNN on 8 Trainium2 NeuronCores (Bass/Tile).

Strategy (per spec sharding_hint, adapted):
  - Nodes are range-sharded across the 8 cores (6250 nodes / 100K edges each);
    dst is the regular repeat(arange(N), 16) pattern so every segment op
    becomes a dense 16-way reduction fully local to the owning core.
  - Phase 1: each core computes embW = emb_shard @ [W1|Vl|Vr] (f32 matmul),
    rounds to bf16, and AllGathers the 50000-row feature table T1 so every
    core can gather arbitrary src rows locally.
  - Conv1: per 128-node block, indirect-DMA gather of 2048 src rows
    (520B each), edge softmax without max-subtraction (shift-invariant),
    alpha-weighted segment sum via TensorEngine matmuls against a constant
    block-diagonal selector S' (32 nodes x 4 edge-slots per matmul,
    quartet accumulation in PSUM).
  - h1 -> f2 = h1 @ [W2|V2l|V2r] via PE transpose + matmul, AllGather T2,
    Conv2 identical structure with 1 head.
  - Final 64x64 score/loss epilogue computed on host from gathered h2.
"""

import sys
import numpy as np

sys.path.insert(0, "/opt/trn_rl_repo")

# problem constants
N = 50000
DEG = 16
E = N * DEG
IN = 128
HID = 64
H = 4
B = 64
NEG_SLOPE = 0.2
NCORES = 8
NS = N // NCORES            # 6250 nodes per core
NBLK = (NS + 127) // 128    # 49 blocks
NPAD = NBLK * 128           # 6272
T1W = 132                   # f32 cols/row: [0:128]=f dmajor bf16x256, [128:130]=el bf16x4, [130:132]=er bf16x4
T2W = 34                    # f32 cols/row: [0:32]=f2 bf16x64, [32]=el2+pad, [33]=er2+pad


def _leaky(x, s=NEG_SLOPE):
    return np.where(x > 0, x, s * x)


def _numpy_ref(feat_ids, src, dst, user_ids, item_ids, emb,
               W1, a_l1, a_r1, b1, W2, a_l2, a_r2, b2):
    """General fallback (any sorted dst) — pure numpy."""
    def gat(feat, W, a_l, a_r, b, src, dst, n):
        nh, d = a_l.shape
        f = (feat @ W).reshape(n, nh, d)
        el = (f * a_l[None]).sum(-1)
        er = (f * a_r[None]).sum(-1)
        e = _leaky(el[src] + er[dst])
        m = np.full((n, nh), -np.inf, np.float32)
        np.maximum.at(m, dst, e)
        ex = np.exp(e - m[dst])
        den = np.zeros((n, nh), np.float32)
        np.add.at(den, dst, ex)
        alpha = ex / den[dst]
        out = np.zeros((n, nh, d), np.float32)
        np.add.at(out, dst, alpha[:, :, None] * f[src])
        return out + b[None]

    feats = emb[feat_ids]
    h1 = gat(feats, W1, a_l1, a_r1, b1, src, dst, N).reshape(N, H * HID)
    h1 = np.maximum(h1, 0)
    h2 = gat(h1, W2, a_l2, a_r2, b2, src, dst, N).mean(axis=1)
    return _host_epilogue(h2, user_ids, item_ids)


def _host_epilogue(h2, user_ids, item_ids):
    ue = h2[user_ids].astype(np.float32)
    ie = h2[item_ids].astype(np.float32)
    scores = ue * ie
    labels = np.eye(B, dtype=np.float32)
    m = scores.max(axis=-1, keepdims=True)
    lse = m[:, 0] + np.log(np.exp(scores - m).sum(axis=-1))
    loss = np.float32(np.mean(lse - np.diag(scores)))
    return (loss, scores, labels)


def _build_nc(bias_zero):
    import concourse.bass as bass
    import concourse.mybir as mybir
    from concourse import bacc, tile
    from concourse.masks import make_identity

    dt = mybir.dt
    Alu = mybir.AluOpType
    Act = mybir.ActivationFunctionType

    nc = bacc.Bacc(None, target_bir_lowering=False, debug=False)

    # ---- external inputs (per-core shards / replicated consts) ----
    embT = nc.declare_dram_parameter("embT", [IN, NPAD], dt.float32, isOutput=False)
    wcat1 = nc.declare_dram_parameter("wcat1", [IN, 264], dt.float32, isOutput=False)
    wcat2 = nc.declare_dram_parameter("wcat2", [128, 132], dt.bfloat16, isOutput=False)
    sprime = nc.declare_dram_parameter("sprime", [128, 32], dt.bfloat16, isOutput=False)
    big1 = nc.declare_dram_parameter("big1", [NBLK, 128, 16], dt.int32, isOutput=False)
    er1i = nc.declare_dram_parameter("er1i", [NBLK, 128, 16], dt.int32, isOutput=False)
    big2 = nc.declare_dram_parameter("big2", [NBLK, 128, 16], dt.int32, isOutput=False)
    er2i = nc.declare_dram_parameter("er2i", [NBLK, 128, 16], dt.int32, isOutput=False)
    if not bias_zero:
        b1d = nc.declare_dram_parameter("b1d", [1, 256], dt.float32, isOutput=False)
        b2d = nc.declare_dram_parameter("b2d", [1, 64], dt.float32, isOutput=False)
    out_ext = nc.declare_dram_parameter("out", [NS, HID], dt.float32, isOutput=True)

    with tile.TileContext(nc) as tc:
        with (
            tc.tile_pool(name="dram", bufs=1, space="DRAM") as dram,
            tc.tile_pool(name="consts", bufs=1) as consts,
            tc.tile_pool(name="h1pool", bufs=1) as h1pool,
            tc.tile_pool(name="p1", bufs=3) as p1,
            tc.tile_pool(name="gpool", bufs=3) as gpool,
            tc.tile_pool(name="spool", bufs=3) as spool,
            tc.tile_pool(name="cps", bufs=4, space="PSUM") as cps,
        ):
            # ---- persistent constants in SBUF ----
            sp_sb = consts.tile([128, 32], dt.bfloat16)
            nc.sync.dma_start(out=sp_sb[:], in_=sprime[:])
            wc1_sb = consts.tile([IN, 264], dt.float32)
            nc.sync.dma_start(out=wc1_sb[:], in_=wcat1[:])
            wc2_sb = consts.tile([128, 132], dt.bfloat16)
            nc.sync.dma_start(out=wc2_sb[:], in_=wcat2[:])
            ident = consts.tile([128, 128], dt.bfloat16)
            make_identity(nc, ident[:])
            if not bias_zero:
                b1_sb = consts.tile([128, 256], dt.float32)
                nc.sync.dma_start(
                    out=b1_sb[:],
                    in_=bass.AP(b1d[:].tensor, 0, [[0, 128], [1, 256]]))
                b2_sb = consts.tile([128, 64], dt.float32)
                nc.sync.dma_start(
                    out=b2_sb[:],
                    in_=bass.AP(b2d[:].tensor, 0, [[0, 128], [1, 64]]))

            # persistent h1 activations: [p, (blk, c)] bf16
            h1_t = h1pool.tile([128, NBLK * 256], dt.bfloat16)

            # ---- internal DRAM tables ----
            t1_shard = dram.tile([NS, T1W], dt.float32)
            t1_full = dram.tile([N, T1W], dt.float32, addr_space="Shared")
            t2_shard = dram.tile([NS, T2W], dt.float32)
            t2_full = dram.tile([N, T2W], dt.float32, addr_space="Shared")

            # =========== Phase 1: T1 = [embW | el | er] (f32 matmul) ===========
            for b in range(NBLK):
                rows = min(128, NS - b * 128)
                et = p1.tile([128, 128], dt.float32, tag="et")
                nc.sync.dma_start(out=et[:], in_=embT[:, b * 128:(b + 1) * 128])
                ps = cps.tile([128, 264], dt.float32, tag="ps", padded_shape=[128, 512])
                nc.tensor.matmul(out=ps[:], lhsT=et[:],
                                 rhs=wc1_sb[:], start=True, stop=True)
                asm = p1.tile([128, T1W], dt.float32, tag="asm1")
                asm_bf = asm[:].bitcast(dt.bfloat16)  # [128, 264]
                nc.scalar.activation(asm_bf[:, 0:256], ps[:, 0:256], Act.Copy)
                nc.vector.tensor_copy(asm_bf[:, 256:264], ps[:, 256:264])
                nc.sync.dma_start(out=t1_shard[b * 128:b * 128 + rows, :],
                                  in_=asm[:rows, :])

            nc.gpsimd.collective_compute(
                "AllGather", Alu.bypass,
                replica_groups=[list(range(NCORES))],
                ins=[t1_shard[:].opt()], outs=[t1_full[:].opt()])

            # =========== Conv1 blocks ===========
            for b in range(NBLK):
                i1 = spool.tile([128, 16], dt.int32, tag="i1")
                nc.sync.dma_start(out=i1[:], in_=big1[b])
                ie = spool.tile([128, 16], dt.int32, tag="ie")
                nc.sync.dma_start(out=ie[:], in_=er1i[b])

                G = gpool.tile([128, 16 * 130], dt.float32, tag="G1")
                nc.gpsimd.indirect_dma_start(
                    out=G[:], out_offset=None, in_=t1_full[:],
                    in_offset=bass.IndirectOffsetOnAxis(ap=i1[:], axis=0))
                ert = spool.tile([128, 16 * 2], dt.float32, tag="ert")
                nc.gpsimd.indirect_dma_start(
                    out=ert[:], out_offset=None,
                    in_=t1_full[:].rearrange("a (c d) -> (a c) d", d=2),
                    in_offset=bass.IndirectOffsetOnAxis(ap=ie[:], axis=0),
                    element_offset=130)

                Gv = G[:].bitcast(dt.bfloat16)       # [128, 16*260]
                erv = ert[:].bitcast(dt.bfloat16)    # [128, 16*4]
                p0g = Gv.ap[0]
                p0e = erv.ap[0]

                e_t = spool.tile([128, 64], dt.float32, tag="e_t")
                nc.vector.tensor_tensor(
                    out=e_t[:].rearrange("p (g h) -> p g h", h=4),
                    in0=bass.AP(Gv.tensor, Gv.offset + 256, [p0g, [260, 16], [1, 4]]),
                    in1=bass.AP(erv.tensor, erv.offset, [p0e, [4, 16], [1, 4]]),
                    op=Alu.add)
                lk = spool.tile([128, 64], dt.float32, tag="lk")
                nc.vector.tensor_scalar(out=lk[:], in0=e_t[:], scalar1=NEG_SLOPE,
                                        scalar2=None, op0=Alu.mult)
                nc.vector.tensor_tensor(out=e_t[:], in0=e_t[:], in1=lk[:], op=Alu.max)
                ex = spool.tile([128, 64], dt.bfloat16, tag="ex")
                nc.scalar.activation(ex[:], e_t[:], Act.Exp)

                exG = gpool.tile([128, 16 * 260], dt.bfloat16, tag="exG")
                pxg = exG[:].ap[0]
                pex = ex[:].ap[0]
                nc.vector.tensor_tensor(
                    out=bass.AP(exG[:].tensor, exG[:].offset, [pxg, [260, 16], [4, 64], [1, 4]]),
                    in0=bass.AP(Gv.tensor, Gv.offset, [p0g, [260, 16], [4, 64], [1, 4]]),
                    in1=bass.AP(ex[:].tensor, ex[:].offset, [pex, [4, 16], [0, 64], [1, 4]]),
                    op=Alu.mult)
                nc.vector.tensor_copy(
                    bass.AP(exG[:].tensor, exG[:].offset + 256, [pxg, [260, 16], [1, 4]]),
                    bass.AP(ex[:].tensor, ex[:].offset, [pex, [4, 16], [1, 4]]))

                ps1 = cps.tile([128, 260], dt.float32, tag="ps", padded_shape=[128, 512])
                for a in range(4):
                    for q in range(4):
                        g = 4 * a + q
                        rhs = bass.AP(exG[:].tensor, exG[:].offset + g * 260,
                                      [pxg, [1, 260]])
                        nc.tensor.matmul(out=ps1[32 * a:32 * a + 32, :],
                                         lhsT=sp_sb[:], rhs=rhs,
                                         start=(q == 0), stop=(q == 3),
                                         tile_position=(0, 32 * a))

                recip = spool.tile([128, 4], dt.float32, tag="recip")
                nc.vector.reciprocal(
                    recip[:], bass.AP(ps1[:].tensor, ps1[:].offset + 256,
                                      [ps1[:].ap[0], [1, 4]]))
                for h in range(4):
                    src_ap = bass.AP(ps1[:].tensor, ps1[:].offset + h,
                                     [ps1[:].ap[0], [4, 64]])
                    dst_ap = h1_t[:, b * 256 + h * 64: b * 256 + h * 64 + 64]
                    if bias_zero:
                        nc.scalar.activation(dst_ap, src_ap, Act.Relu,
                                             scale=recip[:, h:h + 1])
                    else:
                        tmp = spool.tile([128, 64], dt.float32, tag="ep1tmp")
                        nc.scalar.activation(tmp[:], src_ap, Act.Copy,
                                             scale=recip[:, h:h + 1])
                        nc.vector.tensor_tensor(out=tmp[:], in0=tmp[:],
                                                in1=b1_sb[:, h * 64:h * 64 + 64],
                                                op=Alu.add)
                        nc.vector.tensor_scalar(out=dst_ap, in0=tmp[:], scalar1=0.0,
                                                scalar2=None, op0=Alu.max)

            # =========== T2 build ===========
            for b in range(NBLK):
                rows = min(128, NS - b * 128)
                h1T = p1.tile([128, 256], dt.bfloat16, tag="h1T")
                for j in range(2):
                    pst = cps.tile([128, 128], dt.bfloat16, tag="ps", padded_shape=[128, 1024])
                    nc.tensor.transpose(
                        out=pst[:],
                        in_=h1_t[:, b * 256 + j * 128: b * 256 + (j + 1) * 128],
                        identity=ident[:])
                    nc.scalar.activation(h1T[:, j * 128:(j + 1) * 128], pst[:],
                                         Act.Copy)
                psf = cps.tile([128, 66], dt.float32, tag="ps", padded_shape=[128, 512])
                for j in range(2):
                    nc.tensor.matmul(out=psf[:], lhsT=h1T[:, j * 128:(j + 1) * 128],
                                     rhs=wc2_sb[:, j * 66:(j + 1) * 66],
                                     start=(j == 0), stop=(j == 1))
                asm2 = p1.tile([128, T2W], dt.float32, tag="asm2")
                a2bf = asm2[:].bitcast(dt.bfloat16)
                nc.scalar.activation(a2bf[:, 0:64], psf[:, 0:64], Act.Copy)
                nc.vector.memset(
                    bass.AP(a2bf.tensor, a2bf.offset + 65, [a2bf.ap[0], [2, 2]]), 0.0)
                nc.vector.tensor_copy(
                    bass.AP(a2bf.tensor, a2bf.offset + 64, [a2bf.ap[0], [2, 2]]),
                    psf[:, 64:66])
                nc.sync.dma_start(out=t2_shard[b * 128:b * 128 + rows, :],
                                  in_=asm2[:rows, :])

            nc.gpsimd.collective_compute(
                "AllGather", Alu.bypass,
                replica_groups=[list(range(NCORES))],
                ins=[t2_shard[:].opt()], outs=[t2_full[:].opt()])

            # =========== Conv2 blocks ===========
            for b in range(NBLK):
                i2 = spool.tile([128, 16], dt.int32, tag="i2")
                nc.sync.dma_start(out=i2[:], in_=big2[b])
                ie2 = spool.tile([128, 16], dt.int32, tag="ie2")
                nc.sync.dma_start(out=ie2[:], in_=er2i[b])

                G2 = gpool.tile([128, 16 * 33], dt.float32, tag="G2")
                nc.gpsimd.indirect_dma_start(
                    out=G2[:], out_offset=None, in_=t2_full[:],
                    in_offset=bass.IndirectOffsetOnAxis(ap=i2[:], axis=0))
                er2t = spool.tile([128, 16 * 1], dt.float32, tag="er2t")
                nc.gpsimd.indirect_dma_start(
                    out=er2t[:], out_offset=None,
                    in_=t2_full[:].rearrange("a (c d) -> (a c) d", d=2),
                    in_offset=bass.IndirectOffsetOnAxis(ap=ie2[:], axis=0),
                    element_offset=33)

                G2v = G2[:].bitcast(dt.bfloat16)   # [128, 16*66]
                er2v = er2t[:].bitcast(dt.bfloat16)
                p0g2 = G2v.ap[0]

                e2 = spool.tile([128, 16], dt.float32, tag="e2")
                nc.vector.tensor_tensor(
                    out=e2[:].rearrange("p (g o) -> p g o", o=1),
                    in0=bass.AP(G2v.tensor, G2v.offset + 64, [p0g2, [66, 16], [1, 1]]),
                    in1=bass.AP(er2v.tensor, er2v.offset, [er2v.ap[0], [2, 16], [1, 1]]),
                    op=Alu.add)
                lk2 = spool.tile([128, 16], dt.float32, tag="lk2")
                nc.vector.tensor_scalar(out=lk2[:], in0=e2[:], scalar1=NEG_SLOPE,
                                        scalar2=None, op0=Alu.mult)
                nc.vector.tensor_tensor(out=e2[:], in0=e2[:], in1=lk2[:], op=Alu.max)
                ex2 = spool.tile([128, 16], dt.bfloat16, tag="ex2")
                nc.scalar.activation(ex2[:], e2[:], Act.Exp)

                exG2 = gpool.tile([128, 16 * 66], dt.bfloat16, tag="exG2")
                px2 = exG2[:].ap[0]
                pe2 = ex2[:].ap[0]
                nc.vector.tensor_tensor(
                    out=bass.AP(exG2[:].tensor, exG2[:].offset, [px2, [66, 16], [1, 64]]),
                    in0=bass.AP(G2v.tensor, G2v.offset, [p0g2, [66, 16], [1, 64]]),
                    in1=bass.AP(ex2[:].tensor, ex2[:].offset, [pe2, [1, 16], [0, 64]]),
                    op=Alu.mult)
                nc.vector.tensor_copy(
                    bass.AP(exG2[:].tensor, exG2[:].offset + 64, [px2, [66, 16], [1, 1]]),
                    bass.AP(ex2[:].tensor, ex2[:].offset, [pe2, [1, 16], [1, 1]]))

                ps2 = cps.tile([128, 65], dt.float32, tag="ps", padded_shape=[128, 512])
                for a in range(4):
                    for q in range(4):
                        g = 4 * a + q
                        rhs = bass.AP(exG2[:].tensor, exG2[:].offset + g * 66,
                                      [px2, [1, 65]])
                        nc.tensor.matmul(out=ps2[32 * a:32 * a + 32, :],
                                         lhsT=sp_sb[:], rhs=rhs,
                                         start=(q == 0), stop=(q == 3),
                                         tile_position=(0, 32 * a))

                recip2 = spool.tile([128, 1], dt.float32, tag="recip2")
                nc.vector.reciprocal(recip2[:], ps2[:, 64:65])
                h2sb = spool.tile([128, 64], dt.float32, tag="h2sb")
                nc.scalar.activation(h2sb[:], ps2[:, 0:64], Act.Copy,
                                     scale=recip2[:, 0:1])
                if not bias_zero:
                    nc.vector.tensor_tensor(out=h2sb[:], in0=h2sb[:], in1=b2_sb[:],
                                            op=Alu.add)
                rows = min(128, NS - b * 128)
                nc.sync.dma_start(out=out_ext[b * 128:b * 128 + rows, :],
                                  in_=h2sb[:rows, :])

    nc.compile()
    return nc


def _host_prep(feat_ids, src, emb, W1, a_l1, a_r1, W2, a_l2, a_r2, b1, b2):
    import ml_dtypes
    bf16 = ml_dtypes.bfloat16
    f32 = np.float32

    feat_ids = np.asarray(feat_ids).astype(np.int64)
    src = np.asarray(src).astype(np.int64)
    emb = np.asarray(emb, dtype=f32)
    W1 = np.asarray(W1, dtype=f32)
    W2 = np.asarray(W2, dtype=f32)

    # d-major W1 + attention projections
    W1r = W1.reshape(IN, H, HID)                      # [128, h, d]
    W1dm = W1r.transpose(0, 2, 1).reshape(IN, 256)    # col = d*4+h
    Vl1 = np.einsum("ihd,hd->ih", W1r, np.asarray(a_l1, f32))   # [128, 4]
    Vr1 = np.einsum("ihd,hd->ih", W1r, np.asarray(a_r1, f32))
    wcat1 = np.concatenate([W1dm, Vl1, Vr1], axis=1).astype(f32)  # [128, 264]

    V2l = (W2 @ np.asarray(a_l2, f32)[0]).reshape(256, 1)
    V2r = (W2 @ np.asarray(a_r2, f32)[0]).reshape(256, 1)
    wcat2_tall = np.concatenate([W2, V2l, V2r], axis=1)           # [256, 66]
    wcat2 = np.concatenate([wcat2_tall[:128], wcat2_tall[128:]], axis=1).astype(bf16)  # [128, 132]

    sprime = np.zeros((128, 32), dtype=bf16)
    p = np.arange(128)
    sprime[p, p // 4] = 1

    # slot mapping: block b, partition p, group g
    pg_p = np.arange(128)[:, None]        # [128,1]
    pg_g = np.arange(16)[None, :]         # [1,16]
    nib = 32 * (pg_g // 4) + pg_p // 4    # [128,16] node-in-block
    kk = 4 * (pg_g % 4) + pg_p % 4        # [128,16]

    in_maps = []
    for c in range(NCORES):
        node_local = (np.arange(NBLK)[:, None, None] * 128 + nib[None])  # [NBLK,128,16]
        valid = node_local < NS
        nl = np.where(valid, node_local, 0)
        node_glob = c * NS + nl
        eid = DEG * node_glob + kk[None]
        sfi = feat_ids[src[eid]]
        big1 = np.where(valid, sfi, 0).astype(np.int32)
        er1 = np.where(valid, 66 * feat_ids[node_glob], 0).astype(np.int32)
        big2 = np.where(valid, src[eid], 0).astype(np.int32)
        er2 = np.where(valid, 17 * node_glob, 0).astype(np.int32)

        embT = np.zeros((IN, NPAD), dtype=f32)
        embT[:, :NS] = emb[c * NS:(c + 1) * NS].T
        in_maps.append({
            "embT": embT, "wcat1": wcat1, "wcat2": wcat2, "sprime": sprime,
            "big1": big1, "er1i": er1, "big2": big2, "er2i": er2,
        })
    bias_zero = (not np.any(b1)) and (not np.any(b2))
    if not bias_zero:
        b1dm = np.asarray(b1, f32).reshape(H, HID)
        b1row = b1dm.reshape(1, 256).copy()  # h-major (epilogue writes h-major)
        b2row = np.asarray(b2, f32).reshape(1, 64).copy()
        for m in in_maps:
            m["b1d"] = b1row
            m["b2d"] = b2row
    return in_maps, bias_zero


_CACHE = {}


def _get_runner(bias_zero):
    """Build nc once and keep a reusable jitted runner."""
    key = bias_zero
    if key not in _CACHE:
        nc = _build_nc(bias_zero)
        _CACHE[key] = nc
    return _CACHE[key]


def kernel(**inputs):
    feat_ids = np.asarray(inputs["feat_ids"])
    src = np.asarray(inputs["src"])
    dst = np.asarray(inputs["dst"])
    user_ids = np.asarray(inputs["user_ids"]).astype(np.int64)
    item_ids = np.asarray(inputs["item_ids"]).astype(np.int64)

    regular = bool(np.array_equal(dst, np.repeat(np.arange(N, dtype=dst.dtype), DEG)))
    if not regular:
        return _numpy_ref(**{k: np.asarray(v) for k, v in inputs.items()})

    in_maps, bias_zero = _host_prep(
        feat_ids, src, inputs["emb"], inputs["W1"], inputs["a_l1"], inputs["a_r1"],
        inputs["W2"], inputs["a_l2"], inputs["a_r2"], inputs["b1"], inputs["b2"])

    from concourse.bass_utils import run_bass_kernel_spmd
    nc = _get_runner(bias_zero)
    res = run_bass_kernel_spmd(nc, in_maps, list(range(NCORES)))
    h2 = np.concatenate([res.results[c]["out"] for c in range(NCORES)], axis=0)
    return _host_epilogue(h2, user_ids, item_ids)
